# revision 15
# baseline (speedup 1.0000x reference)
"""Trainium2 Bass kernel for the argmax-distance-weighted loss.

loss = sum_b sum_{j,k} ((jstar_b - j)^2 + (kstar_b - k)^2) * t[b,j,k]
where (jstar_b, kstar_b) is the (first-occurrence) argmax location of t[b].

Decomposition used per batch b:
    loss_b = (jstar^2 + kstar^2)*S - 2*jstar*Sj - 2*kstar*Sk + Sj2 + Sk2
with S    = sum t[b]
     Sj   = sum_j j   * rowsum[b, j]      rowsum[b,j] = sum_k t[b,j,k]
     Sj2  = sum_j j^2 * rowsum[b, j]
     Sk   = sum_k k   * colsum[b, k]      colsum[b,k] = sum_j t[b,j,k]
     Sk2  = sum_k k^2 * colsum[b, k]

Device (8 NeuronCores, data-parallel over batch): per 128-batch tile the
DVE does three full reduction passes (rowsum, colsum contiguous/strided,
rowmax) plus tiny fused weighted reductions, emitting 8 moments per batch.
jstar is recovered exactly (first row whose rowmax equals the batch max).
Host: gathers row jstar per batch (64 floats) to resolve kstar with exact
first-occurrence semantics, then evaluates the closed form and sums.
"""

import os
import sys

import numpy as np

try:
    import concourse.bass as bass
except ModuleNotFoundError:  # make concourse importable in a bare container
    for _p in ("/opt/trn_rl_repo", "/root/.axon_site/_ro/trn_rl_repo"):
        if os.path.isdir(_p) and _p not in sys.path:
            sys.path.insert(0, _p)
    import concourse.bass as bass

import concourse.mybir as mybir
from concourse.bass_utils import run_bass_kernel_spmd
from concourse.tile import TileContext
# --- workaround: this walrus build encodes only ONE sync-wait per TPB ---
# instruction. Tile attaches several waits to one instruction (tail drain,
# DMA copies, ...), which codegen rejects with "Too many sync wait
# commands". Post-pass: hoist all but the last wait of each instruction
# into standalone same-engine NoOps placed immediately before it.


def _split_multiwait_instructions(nc: bass.Bass) -> None:
    # (bb, inst-name) pairs needing surgery
    targets = []
    for fn in nc.m.functions:
        for bb in fn.blocks:
            for inst in bb.instructions:
                si = inst.sync_info
                if si is not None and len(si.on_wait) > 1:
                    targets.append((bb, inst.name))
    if not targets:
        return

    moved_nop_names: set[str] = set()
    plan: dict[str, list] = {}  # target-inst-name -> nop instructions
    for bb, iname in targets:
        inst = next(i for i in bb.instructions if i.name == iname)
        waits = list(inst.sync_info.on_wait)
        inst.sync_info.on_wait = waits[-1:]
        nops = []
        for w in waits[:-1]:
            bi = nc.engines[inst.engine].nop(nofuse=True, hint="split_wait")
            bi.ins.sync_info = mybir.SyncInfo(on_wait=[w], on_update=[])
            nops.append(bi.ins)
            moved_nop_names.add(bi.ins.name)
        plan[iname] = nops

    # relocate the nops to sit immediately before their target instruction
    for fn in nc.m.functions:
        for bb in fn.blocks:
            insts = list(bb.instructions)
            kept = [i for i in insts if i.name not in moved_nop_names]
            out: list = []
            changed = len(kept) != len(insts)
            for inst in kept:
                if inst.name in plan:
                    out.extend(plan[inst.name])
                    changed = True
                out.append(inst)
            if changed:
                bb.instructions = out

B, H, W = 8192, 64, 64
NCORES = 8
P = 128  # SBUF partitions

F32 = mybir.dt.float32
Alu = mybir.AluOpType
Ax = mybir.AxisListType

# output layout: quantity-major [P, NQ, ntiles]
Q_M, Q_S, Q_SJ, Q_SJ2, Q_SK, Q_SK2, Q_RJ = range(7)
NQ = 7


def build(bpc: int, repeats: int = 1, gp: bool = True, gp_rs: int = 6) -> bass.Bass:
    """Build the per-core Bass program for `bpc` batches per core.

    `repeats` re-runs the whole pipeline N times in one program — used only
    for timing (slope method cancels the host dispatch overhead).
    `gp` offloads the colsum fold tree and the elementwise muls to GpSimd so
    the DVE runs almost only dedicated-port single-src reductions."""
    ntiles = bpc // P
    assert ntiles * P == bpc
    NT = ntiles

    nc = bass.Bass()
    x = nc.declare_dram_parameter("x", [bpc, H, W], F32, isOutput=False)
    wc = nc.declare_dram_parameter("wconsts", [3, NT, W], F32, isOutput=False)
    out = nc.declare_dram_parameter("moments", [P, NQ * NT], F32, isOutput=True)

    with TileContext(nc) as tc:
        with (
            tc.tile_pool(name="xpool", bufs=3) as xpool,
            tc.tile_pool(name="folds", bufs=2) as fpool,
            tc.tile_pool(name="consts", bufs=1) as cpool,
            tc.tile_pool(name="inter", bufs=1) as ipool,
        ):
            # broadcast weight constants [3, NT, W] across all partitions
            wtile = cpool.tile([P, 3, NT, W], F32)
            wc_ap = wc[:, :, :]
            bcast = bass.AP(
                tensor=wc_ap.tensor,
                offset=wc_ap.offset,
                ap=[[0, P]] + list(wc_ap.ap),
            )
            nc.sync.dma_start(out=wtile, in_=bcast)
            w1 = wtile[:, 0, :, :]  # [P, NT, W] = j (0..63), tiled per tile
            wr = wtile[:, 2, :, :]  # [P, NT, W] = 64-j

            rs_all = ipool.tile([P, NT, W], F32)
            cs_all = ipool.tile([P, NT, W], F32)
            rm_all = ipool.tile([P, NT, W], F32)
            scrA = ipool.tile([P, NT, W], F32)
            scrB = ipool.tile([P, NT, W], F32)
            outq = ipool.tile([P, NQ * NT], F32)

            def O(q):
                return outq[:, q * NT : (q + 1) * NT]

            eng = nc.gpsimd if gp else nc.vector

            for rep in range(repeats):
                for t in range(ntiles):
                    xt = xpool.tile([P, H, W], F32)
                    nchunk = 4 if t == 0 else 1
                    hs = H // nchunk
                    for c in range(nchunk):
                        nc.sync.dma_start(
                            out=xt[:, c * hs : (c + 1) * hs, :],
                            in_=x[t * P : (t + 1) * P, c * hs : (c + 1) * hs, :],
                        )

                    # rowmax always on DVE (single-src, dedicated port)
                    for c in range(nchunk):
                        nc.vector.tensor_reduce(
                            out=rm_all[:, t, c * hs : (c + 1) * hs],
                            in_=xt[:, c * hs : (c + 1) * hs, :], axis=Ax.X, op=Alu.max,
                        )
                    rs_on_gp = gp and (t % 8) not in (0, 4) and (t % 8) < gp_rs + 2
                    if not rs_on_gp:
                        for c in range(nchunk):
                            nc.vector.tensor_reduce(
                                out=rs_all[:, t, c * hs : (c + 1) * hs],
                                in_=xt[:, c * hs : (c + 1) * hs, :], axis=Ax.X, op=Alu.add,
                            )
                    else:
                        # rowsum as a k-fold tree on GpSimd (strided halves
                        # within each row): view [P, 64, w] -> [P, 64, w/2]
                        foldr = fpool.tile([P, 4096], F32, tag="foldr")

                        def v3(ap_flat, wdt):
                            return ap_flat.rearrange("p (a b) -> p a b", a=H, b=wdt)

                        cur = xt[:, :, :]
                        cur_w = W
                        offs = [0, 2048, 3072, 3584, 3840]
                        outs_seg = []
                        for li in range(5):
                            half = cur_w // 2
                            dst = v3(foldr[:, offs[li] : offs[li] + H * half], half)
                            nc.gpsimd.tensor_tensor(
                                out=dst, in0=cur[:, :, 0:half],
                                in1=cur[:, :, half:cur_w], op=Alu.add,
                            )
                            cur, cur_w = dst, half
                        nc.gpsimd.tensor_tensor(
                            out=rs_all[:, t, :], in0=cur[:, :, 0:1].rearrange("p a b -> p (a b)"),
                            in1=cur[:, :, 1:2].rearrange("p a b -> p (a b)"), op=Alu.add,
                        )
                    if gp:
                        # colsum over j as a fold tree on flat contiguous
                        # halves: out[i] = in[i] + in[i+half] (k stays the
                        # innermost 64)
                        xf = xt[:, :, :].rearrange("p a b -> p (a b)")
                        fold = fpool.tile([P, 4096], F32, tag="fold")
                        if t == 0:
                            # two half-width L0 folds so GP starts after the
                            # first two DMA chunks instead of all four
                            nc.gpsimd.tensor_tensor(
                                out=fold[:, 0:1024], in0=xf[:, 0:1024],
                                in1=xf[:, 2048:3072], op=Alu.add,
                            )
                            nc.gpsimd.tensor_tensor(
                                out=fold[:, 1024:2048], in0=xf[:, 1024:2048],
                                in1=xf[:, 3072:4096], op=Alu.add,
                            )
                            seg = [(2048, 1024), (3072, 512),
                                   (3584, 256), (3840, 128)]
                            src, src_off = fold, 0
                        else:
                            seg = [(0, 2048), (2048, 1024), (3072, 512),
                                   (3584, 256), (3840, 128)]
                            src, src_off = xf, 0
                        for (dst_off, dst_n) in seg:
                            nc.gpsimd.tensor_tensor(
                                out=fold[:, dst_off : dst_off + dst_n],
                                in0=src[:, src_off : src_off + dst_n],
                                in1=src[:, src_off + dst_n : src_off + 2 * dst_n],
                                op=Alu.add,
                            )
                            src, src_off = fold, dst_off
                        nc.gpsimd.tensor_tensor(
                            out=cs_all[:, t, :], in0=fold[:, 3840:3904],
                            in1=fold[:, 3904:3968], op=Alu.add,
                        )
                    else:
                        xk = xt[:, :, :].rearrange("p j k -> p k j")
                        nc.vector.tensor_reduce(
                            out=cs_all[:, t, :], in_=xk, axis=Ax.X, op=Alu.add
                        )

                # weighted sums per tile-half (overlap with streaming)
                halves = [(0, NT // 2), (NT // 2, NT)] if NT >= 2 else [(0, NT)]
                for (h0, h1) in halves:
                    hv = lambda a: a[:, h0:h1, :]
                    hq = lambda q: outq[:, q * NT + h0 : q * NT + h1]
                    eng.tensor_tensor(out=hv(scrA), in0=hv(rs_all), in1=hv(w1), op=Alu.mult)
                    nc.vector.tensor_reduce(out=hq(Q_SJ), in_=hv(scrA), axis=Ax.X, op=Alu.add)
                    eng.tensor_tensor(out=hv(scrB), in0=hv(scrA), in1=hv(w1), op=Alu.mult)
                    nc.vector.tensor_reduce(out=hq(Q_SJ2), in_=hv(scrB), axis=Ax.X, op=Alu.add)
                    eng.tensor_tensor(out=hv(scrA), in0=hv(cs_all), in1=hv(w1), op=Alu.mult)
                    nc.vector.tensor_reduce(out=hq(Q_SK), in_=hv(scrA), axis=Ax.X, op=Alu.add)
                    eng.tensor_tensor(out=hv(scrB), in0=hv(scrA), in1=hv(w1), op=Alu.mult)
                    nc.vector.tensor_reduce(out=hq(Q_SK2), in_=hv(scrB), axis=Ax.X, op=Alu.add)
                    nc.vector.tensor_reduce(out=hq(Q_S), in_=hv(rs_all), axis=Ax.X, op=Alu.add)
                    nc.vector.tensor_reduce(out=hq(Q_M), in_=hv(rm_all), axis=Ax.X, op=Alu.max)
                # jstar: ge = (rm >= M) * (64-j); rj = max; jstar = 64 - rj
                mb = O(Q_M).unsqueeze(2).to_broadcast([P, NT, W])
                nc.vector.tensor_tensor(out=scrB, in0=rm_all, in1=mb, op=Alu.is_ge)
                eng.tensor_tensor(out=scrA, in0=scrB, in1=wr, op=Alu.mult)
                nc.vector.tensor_reduce(out=O(Q_RJ), in_=scrA[:, :, :], axis=Ax.X, op=Alu.max)

            nc.sync.dma_start(out=out[:, :], in_=outq)

    _split_multiwait_instructions(nc)
    return nc


_cache: dict[int, bass.Bass] = {}


def _get(bpc: int) -> bass.Bass:
    if bpc not in _cache:
        _cache[bpc] = build(bpc)
    return _cache[bpc]


def _wconsts(ntiles: int) -> np.ndarray:
    j = np.arange(W, dtype=np.float32)
    base = np.stack([j, j * j, (W - j).astype(np.float32)])  # [3, W]
    return np.repeat(base[:, None, :], ntiles, axis=1)  # [3, NT, W]


def _prepare(tensor: np.ndarray):
    t = np.ascontiguousarray(np.asarray(tensor), dtype=np.float32)
    bt = t.shape[0]
    bpc = bt // NCORES
    nc = _get(bpc)
    wc = _wconsts(bpc // P)
    in_maps = [
        {"x": t[c * bpc : (c + 1) * bpc], "wconsts": wc} for c in range(NCORES)
    ]
    return nc, in_maps, t


def _postprocess(t: np.ndarray, results: list[dict]) -> np.ndarray:
    bt = t.shape[0]
    bpc = bt // NCORES
    nt = bpc // P
    ms = []
    for c in range(NCORES):
        m = results[c]["moments"].reshape(P, NQ, nt)
        ms.append(m.transpose(2, 0, 1).reshape(bpc, NQ))  # batch-major
    m = np.concatenate(ms, 0).astype(np.float64)  # [B, NQ]

    S = m[:, Q_S]
    Sj = m[:, Q_SJ]
    Sj2 = m[:, Q_SJ2]
    Sk = m[:, Q_SK]
    Sk2 = m[:, Q_SK2]
    jstar = np.rint(W - m[:, Q_RJ]).astype(np.int64)

    # resolve kstar with exact first-occurrence semantics on the argmax row
    rows = t[np.arange(bt), jstar, :]  # [B, W]
    mrow = rows.max(axis=1)
    kstar = (rows == mrow[:, None]).argmax(axis=1)

    js = jstar.astype(np.float64)
    ks = kstar.astype(np.float64)
    loss = ((js * js + ks * ks) * S - 2.0 * js * Sj - 2.0 * ks * Sk + Sj2 + Sk2).sum()
    return np.asarray([loss], dtype=np.float32)


def kernel(tensor: np.ndarray) -> np.ndarray:
    nc, in_maps, t = _prepare(tensor)
    res = run_bass_kernel_spmd(nc, in_maps, list(range(NCORES)))
    return _postprocess(t, res.results)


# revision 18
# speedup vs baseline: 1.7749x; 1.7749x over previous
"""Trainium2 Bass kernel for the argmax-distance-weighted loss.

loss = sum_b sum_{j,k} ((jstar_b - j)^2 + (kstar_b - k)^2) * t[b,j,k]
where (jstar_b, kstar_b) is the (first-occurrence) argmax location of t[b].

Decomposition used per batch b:
    loss_b = (jstar^2 + kstar^2)*S - 2*jstar*Sj - 2*kstar*Sk + Sj2 + Sk2
with S    = sum t[b]
     Sj   = sum_j j   * rowsum[b, j]      rowsum[b,j] = sum_k t[b,j,k]
     Sj2  = sum_j j^2 * rowsum[b, j]
     Sk   = sum_k k   * colsum[b, k]      colsum[b,k] = sum_j t[b,j,k]
     Sk2  = sum_k k^2 * colsum[b, k]

Device (8 NeuronCores, data-parallel over batch, 8x [128, 64, 64] tiles per
core): per tile the DVE runs two single-source tensor_reduce passes (rowsum,
rowmax) on its dedicated SBUF port while GpSimd computes colsum concurrently
as a contiguous-halves fold tree (out[i] = in[i] + in[i+half], which folds j
away and keeps k innermost) on the DVE/GpSimd shared port. A batched
epilogue derives M, S, Sj, Sj2, Sk, Sk2 and rj per batch; jstar = 64 - rj is
exact (first row whose rowmax equals the batch max, via an is_ge mask times
a reversed-index weight, max-reduced). Engines balance at ~70us (DVE) /
~32us (Pool) / ~53us (DMA) per core; steady-state ~72us, single-shot ~84us
vs a ~47us HBM roofline (16.8 MB/core at ~358 GB/s).

Host (the "gather/unshard" step): gathers row jstar per batch (64 floats,
0.4% of the data) to resolve kstar with exact first-occurrence tie
semantics — matching jnp.argmax's flat scan order exactly, since the first
flat maximum is (first row containing M, first k with M within that row) —
then evaluates the closed form in float64 and sums. The tie handling is
load-bearing: the actual jax.random input has batches with duplicated
maxima.

Toolchain notes (this container's pinned walrus build):
- only ONE sync-wait is encodable per TPB instruction; Tile attaches
  several (tail drain, DMA copies) -> _split_multiwait_instructions
  post-pass hoists extras into standalone same-engine NoOps.
- InstTensorTensorReduce ("ISA wrong length"), InstPool (verifier assert),
  and TensorScalarPtr-on-Pool ("engine check failed") are all unusable;
  Pool accepts only arithmetic InstTensorTensor (add/mult, no is_ge).
- GpSimd throughput: contiguous tensor_tensor streams fast (~0.7 ns/elem),
  strided APs are several times slower on real HW than the cost model says.
"""

import os
import sys

import numpy as np

try:
    import concourse.bass as bass
except ModuleNotFoundError:  # make concourse importable in a bare container
    for _p in ("/opt/trn_rl_repo", "/root/.axon_site/_ro/trn_rl_repo"):
        if os.path.isdir(_p) and _p not in sys.path:
            sys.path.insert(0, _p)
    import concourse.bass as bass

import concourse.mybir as mybir
from concourse.bass_utils import run_bass_kernel_spmd
from concourse.tile import TileContext
# --- workaround: this walrus build encodes only ONE sync-wait per TPB ---
# instruction. Tile attaches several waits to one instruction (tail drain,
# DMA copies, ...), which codegen rejects with "Too many sync wait
# commands". Post-pass: hoist all but the last wait of each instruction
# into standalone same-engine NoOps placed immediately before it.


def _split_multiwait_instructions(nc: bass.Bass) -> None:
    # (bb, inst-name) pairs needing surgery
    targets = []
    for fn in nc.m.functions:
        for bb in fn.blocks:
            for inst in bb.instructions:
                si = inst.sync_info
                if si is not None and len(si.on_wait) > 1:
                    targets.append((bb, inst.name))
    if not targets:
        return

    moved_nop_names: set[str] = set()
    plan: dict[str, list] = {}  # target-inst-name -> nop instructions
    for bb, iname in targets:
        inst = next(i for i in bb.instructions if i.name == iname)
        waits = list(inst.sync_info.on_wait)
        inst.sync_info.on_wait = waits[-1:]
        nops = []
        for w in waits[:-1]:
            bi = nc.engines[inst.engine].nop(nofuse=True, hint="split_wait")
            bi.ins.sync_info = mybir.SyncInfo(on_wait=[w], on_update=[])
            nops.append(bi.ins)
            moved_nop_names.add(bi.ins.name)
        plan[iname] = nops

    # relocate the nops to sit immediately before their target instruction
    for fn in nc.m.functions:
        for bb in fn.blocks:
            insts = list(bb.instructions)
            kept = [i for i in insts if i.name not in moved_nop_names]
            out: list = []
            changed = len(kept) != len(insts)
            for inst in kept:
                if inst.name in plan:
                    out.extend(plan[inst.name])
                    changed = True
                out.append(inst)
            if changed:
                bb.instructions = out

B, H, W = 8192, 64, 64
NCORES = 8
P = 128  # SBUF partitions

F32 = mybir.dt.float32
Alu = mybir.AluOpType
Ax = mybir.AxisListType

# output layout: quantity-major [P, NQ, ntiles]
Q_M, Q_S, Q_SJ, Q_SJ2, Q_SK, Q_SK2, Q_RJ = range(7)
NQ = 7


def build(bpc: int, repeats: int = 1, gp: bool = True, gp_rs: int = 0) -> bass.Bass:
    """Build the per-core Bass program for `bpc` batches per core.

    `repeats` re-runs the whole pipeline N times in one program — used only
    for timing (slope method cancels the host dispatch overhead).
    `gp` offloads the colsum fold tree and the elementwise muls to GpSimd so
    the DVE runs almost only dedicated-port single-src reductions."""
    ntiles = bpc // P
    assert ntiles * P == bpc
    NT = ntiles

    nc = bass.Bass()
    x = nc.declare_dram_parameter("x", [bpc, H, W], F32, isOutput=False)
    wc = nc.declare_dram_parameter("wconsts", [3, NT, W], F32, isOutput=False)
    out = nc.declare_dram_parameter("moments", [P, NQ * NT], F32, isOutput=True)

    with TileContext(nc) as tc:
        with (
            tc.tile_pool(name="xpool", bufs=3) as xpool,
            tc.tile_pool(name="folds", bufs=2) as fpool,
            tc.tile_pool(name="consts", bufs=1) as cpool,
            tc.tile_pool(name="inter", bufs=1) as ipool,
        ):
            # broadcast weight constants [3, NT, W] across all partitions
            wtile = cpool.tile([P, 3, NT, W], F32)
            wc_ap = wc[:, :, :]
            bcast = bass.AP(
                tensor=wc_ap.tensor,
                offset=wc_ap.offset,
                ap=[[0, P]] + list(wc_ap.ap),
            )
            nc.sync.dma_start(out=wtile, in_=bcast)
            w1 = wtile[:, 0, :, :]  # [P, NT, W] = j (0..63), tiled per tile
            wr = wtile[:, 2, :, :]  # [P, NT, W] = 64-j

            rs_all = ipool.tile([P, NT, W], F32)
            cs_all = ipool.tile([P, NT, W], F32)
            rm_all = ipool.tile([P, NT, W], F32)
            scrA = ipool.tile([P, NT, W], F32)
            scrB = ipool.tile([P, NT, W], F32)
            outq = ipool.tile([P, NQ * NT], F32)

            def O(q):
                return outq[:, q * NT : (q + 1) * NT]

            eng = nc.gpsimd if gp else nc.vector

            for rep in range(repeats):
                for t in range(ntiles):
                    xt = xpool.tile([P, H, W], F32)
                    nchunk = 4 if t == 0 else 1
                    hs = H // nchunk
                    for c in range(nchunk):
                        nc.sync.dma_start(
                            out=xt[:, c * hs : (c + 1) * hs, :],
                            in_=x[t * P : (t + 1) * P, c * hs : (c + 1) * hs, :],
                        )

                    # rowmax always on DVE (single-src, dedicated port)
                    for c in range(nchunk):
                        nc.vector.tensor_reduce(
                            out=rm_all[:, t, c * hs : (c + 1) * hs],
                            in_=xt[:, c * hs : (c + 1) * hs, :], axis=Ax.X, op=Alu.max,
                        )
                    hybrid = gp and ((t % 8) % 4 != 0) and gp_rs > 0
                    if not hybrid:
                        for c in range(nchunk):
                            nc.vector.tensor_reduce(
                                out=rs_all[:, t, c * hs : (c + 1) * hs],
                                in_=xt[:, c * hs : (c + 1) * hs, :], axis=Ax.X, op=Alu.add,
                            )
                    else:
                        # hybrid rowsum: GP folds the k-halves once
                        # (contiguous 128B runs), DVE reduces the remainder
                        foldr = fpool.tile([P, 2048], F32, tag="foldr")
                        fr = foldr[:, :].rearrange("p (a b) -> p a b", a=H, b=W // 2)
                        nc.gpsimd.tensor_tensor(
                            out=fr, in0=xt[:, :, 0 : W // 2],
                            in1=xt[:, :, W // 2 : W], op=Alu.add,
                        )
                        nc.vector.tensor_reduce(
                            out=rs_all[:, t, :], in_=fr, axis=Ax.X, op=Alu.add,
                        )
                    if gp:
                        # colsum over j as a fold tree on flat contiguous
                        # halves: out[i] = in[i] + in[i+half] (k stays the
                        # innermost 64)
                        xf = xt[:, :, :].rearrange("p a b -> p (a b)")
                        fold = fpool.tile([P, 4096], F32, tag="fold")
                        if t == 0:
                            # two half-width L0 folds so GP starts after the
                            # first two DMA chunks instead of all four
                            nc.gpsimd.tensor_tensor(
                                out=fold[:, 0:1024], in0=xf[:, 0:1024],
                                in1=xf[:, 2048:3072], op=Alu.add,
                            )
                            nc.gpsimd.tensor_tensor(
                                out=fold[:, 1024:2048], in0=xf[:, 1024:2048],
                                in1=xf[:, 3072:4096], op=Alu.add,
                            )
                            seg = [(2048, 1024), (3072, 512),
                                   (3584, 256), (3840, 128)]
                            src, src_off = fold, 0
                        else:
                            seg = [(0, 2048), (2048, 1024), (3072, 512),
                                   (3584, 256), (3840, 128)]
                            src, src_off = xf, 0
                        for (dst_off, dst_n) in seg:
                            nc.gpsimd.tensor_tensor(
                                out=fold[:, dst_off : dst_off + dst_n],
                                in0=src[:, src_off : src_off + dst_n],
                                in1=src[:, src_off + dst_n : src_off + 2 * dst_n],
                                op=Alu.add,
                            )
                            src, src_off = fold, dst_off
                        nc.gpsimd.tensor_tensor(
                            out=cs_all[:, t, :], in0=fold[:, 3840:3904],
                            in1=fold[:, 3904:3968], op=Alu.add,
                        )
                    else:
                        xk = xt[:, :, :].rearrange("p j k -> p k j")
                        nc.vector.tensor_reduce(
                            out=cs_all[:, t, :], in_=xk, axis=Ax.X, op=Alu.add
                        )

                # weighted sums per tile-half (overlap with streaming)
                halves = [(0, NT // 2), (NT // 2, NT)] if NT >= 2 else [(0, NT)]
                for (h0, h1) in halves:
                    hv = lambda a: a[:, h0:h1, :]
                    hq = lambda q: outq[:, q * NT + h0 : q * NT + h1]
                    eng.tensor_tensor(out=hv(scrA), in0=hv(rs_all), in1=hv(w1), op=Alu.mult)
                    nc.vector.tensor_reduce(out=hq(Q_SJ), in_=hv(scrA), axis=Ax.X, op=Alu.add)
                    eng.tensor_tensor(out=hv(scrB), in0=hv(scrA), in1=hv(w1), op=Alu.mult)
                    nc.vector.tensor_reduce(out=hq(Q_SJ2), in_=hv(scrB), axis=Ax.X, op=Alu.add)
                    eng.tensor_tensor(out=hv(scrA), in0=hv(cs_all), in1=hv(w1), op=Alu.mult)
                    nc.vector.tensor_reduce(out=hq(Q_SK), in_=hv(scrA), axis=Ax.X, op=Alu.add)
                    eng.tensor_tensor(out=hv(scrB), in0=hv(scrA), in1=hv(w1), op=Alu.mult)
                    nc.vector.tensor_reduce(out=hq(Q_SK2), in_=hv(scrB), axis=Ax.X, op=Alu.add)
                    nc.vector.tensor_reduce(out=hq(Q_S), in_=hv(rs_all), axis=Ax.X, op=Alu.add)
                    nc.vector.tensor_reduce(out=hq(Q_M), in_=hv(rm_all), axis=Ax.X, op=Alu.max)
                # jstar: ge = (rm >= M) * (64-j); rj = max; jstar = 64 - rj
                mb = O(Q_M).unsqueeze(2).to_broadcast([P, NT, W])
                nc.vector.tensor_tensor(out=scrB, in0=rm_all, in1=mb, op=Alu.is_ge)
                eng.tensor_tensor(out=scrA, in0=scrB, in1=wr, op=Alu.mult)
                nc.vector.tensor_reduce(out=O(Q_RJ), in_=scrA[:, :, :], axis=Ax.X, op=Alu.max)

            nc.sync.dma_start(out=out[:, :], in_=outq)

    _split_multiwait_instructions(nc)
    return nc


_cache: dict[int, bass.Bass] = {}


def _get(bpc: int) -> bass.Bass:
    if bpc not in _cache:
        _cache[bpc] = build(bpc)
    return _cache[bpc]


def _wconsts(ntiles: int) -> np.ndarray:
    j = np.arange(W, dtype=np.float32)
    base = np.stack([j, j * j, (W - j).astype(np.float32)])  # [3, W]
    return np.repeat(base[:, None, :], ntiles, axis=1)  # [3, NT, W]


def _prepare(tensor: np.ndarray):
    t = np.ascontiguousarray(np.asarray(tensor), dtype=np.float32)
    bt = t.shape[0]
    bpc = bt // NCORES
    nc = _get(bpc)
    wc = _wconsts(bpc // P)
    in_maps = [
        {"x": t[c * bpc : (c + 1) * bpc], "wconsts": wc} for c in range(NCORES)
    ]
    return nc, in_maps, t


def _postprocess(t: np.ndarray, results: list[dict]) -> np.ndarray:
    bt = t.shape[0]
    bpc = bt // NCORES
    nt = bpc // P
    ms = []
    for c in range(NCORES):
        m = results[c]["moments"].reshape(P, NQ, nt)
        ms.append(m.transpose(2, 0, 1).reshape(bpc, NQ))  # batch-major
    m = np.concatenate(ms, 0).astype(np.float64)  # [B, NQ]

    S = m[:, Q_S]
    Sj = m[:, Q_SJ]
    Sj2 = m[:, Q_SJ2]
    Sk = m[:, Q_SK]
    Sk2 = m[:, Q_SK2]
    jstar = np.rint(W - m[:, Q_RJ]).astype(np.int64)

    # resolve kstar with exact first-occurrence semantics on the argmax row
    rows = t[np.arange(bt), jstar, :]  # [B, W]
    mrow = rows.max(axis=1)
    kstar = (rows == mrow[:, None]).argmax(axis=1)

    js = jstar.astype(np.float64)
    ks = kstar.astype(np.float64)
    loss = ((js * js + ks * ks) * S - 2.0 * js * Sj - 2.0 * ks * Sk + Sj2 + Sk2).sum()
    return np.asarray([loss], dtype=np.float32)


def kernel(tensor: np.ndarray) -> np.ndarray:
    nc, in_maps, t = _prepare(tensor)
    res = run_bass_kernel_spmd(nc, in_maps, list(range(NCORES)))
    return _postprocess(t, res.results)
